# revision 1
# baseline (speedup 1.0000x reference)
"""Trainium2 Bass kernel for nn_EyeRobotAgent block-sparse ("eye") attention.

Shapes: q,k,v [2, 12, 3456, 32] fp32.  S = 16 time-blocks x 216 feats.
Mask structure (per query block t):
  - all 216 keys of block t are candidates (minus img->img),
  - of each past block t-7..t-1, only 19 keys (m in {0..3, 5..19}) are
    visible (proprio m==4 and img m>=20 keys are never visible in the past),
  - joint queries (m in [4,20)) cannot see past joint keys,
  - img queries (m >= 20) cannot see img keys at all.

Strategy (data-parallel: 24 (b,h) pairs over 8 cores, 3 each):
  Pack per block t a compact key set [216 same | 133 past | 35 pad] = 384.
  The 2-D mask folds into the QK matmul via 3 extra contraction rows
  (rank-1 decomposition of the mask predicates); invalid/pad columns get a
  large negative bias so exp() underflows to 0.  Scores are computed
  transposed [kv, q] so probs can be consumed directly by the PV matmul,
  with a ones-column appended to V producing softmax denominators.
  exp() has no max-subtraction (scores are O(6), fp32-safe).
"""
import numpy as np

import concourse.bass as bass
import concourse.mybir as mybir
import concourse.tile as tile
from concourse import bacc
from concourse.bass_utils import run_bass_kernel_spmd
from concourse.masks import make_identity
from concourse.tile_rust import add_dep_helper

B, H, S, D = 2, 12, 3456, 32
F = 216            # feats_per_t
W = 8              # window_len
T = S // F         # 16 blocks
IMG_START = 20     # F - img_feat_size
JOINT_START = 4    # IMG_START - act_size
PAST_SEL = np.array([0, 1, 2, 3] + list(range(5, 20)))   # 19 per past block
NPAST = 19 * (W - 1)     # 133
KV = 384                 # packed kv per block (216 + 133 + pad)
KAUG = D + 3             # 35 contraction rows (32 d + 3 mask-bias rows)
VA = D + 1               # 33 = v columns + ones column
NEG = np.float32(-30000.0)
SCALE = float(1.0 / np.sqrt(np.float32(D)))
N_CORES = 8
BH_PER_CORE = (B * H) // N_CORES      # 3
NPAIR = T // 2                        # 8 block-pairs per (b,h)

F32 = mybir.dt.float32
BF16 = mybir.dt.float16      # half precision: matmul rate 1 cyc/row, 10-bit mantissa
NP_BF16 = np.float16


# ---------------------------------------------------------------- host packing

def _pack_all(q, k, v):
    """q,k,v: [B,H,S,D] fp32 ->
       qt  [24, KAUG, S]     (augmented Q^T)
       kpt [24, T, KAUG, KV] (augmented packed K^T per block)
       vp  [24, T, KV, VA]   (packed V + ones column per block)"""
    nbh = B * H
    qf = q.reshape(nbh, S, D)
    kf = k.reshape(nbh, S, D)
    vf = v.reshape(nbh, S, D)

    m = np.arange(F)
    is_img_m = (m >= IMG_START).astype(np.float32)
    is_joint_m = ((m >= JOINT_START) & (m < IMG_START)).astype(np.float32)
    qm = np.arange(S) % F

    qt = np.zeros((nbh, KAUG, S), np.float32)
    qt[:, :D] = qf.transpose(0, 2, 1)
    qt[:, 32] = (qm >= IMG_START)
    qt[:, 33] = (qm >= JOINT_START) & (qm < IMG_START)
    qt[:, 34] = 1.0

    kpt = np.zeros((nbh, T, KAUG, KV), np.float32)
    vp = np.zeros((nbh, T, KV, VA), np.float32)
    joint_past_bias = np.tile(NEG * is_joint_m[PAST_SEL], W - 1)  # [133]
    for t in range(T):
        blk = slice(F * t, F * (t + 1))
        kpt[:, t, :D, :F] = kf[:, blk].transpose(0, 2, 1)
        kpt[:, t, 32, :F] = NEG * is_img_m
        vp[:, t, :F, :D] = vf[:, blk]
        vp[:, t, :F, 32] = 1.0
        # past blocks t-7 .. t-1, 19 keys each
        taus = np.arange(t - 7, t)
        rows = (F * taus[:, None] + PAST_SEL[None, :]).reshape(-1)   # [133]
        valid = np.repeat(taus >= 0, 19)                             # [133]
        safe_rows = np.where(valid, rows, 0)
        pc = slice(F, F + NPAST)
        kpt[:, t, :D, pc] = np.where(
            valid[None, None, :], kf[:, safe_rows].transpose(0, 2, 1), 0.0)
        kpt[:, t, 33, pc] = joint_past_bias
        kpt[:, t, 34, pc] = np.where(valid, 0.0, NEG)
        vp[:, t, pc, :D] = np.where(
            valid[None, :, None], vf[:, safe_rows], 0.0)
        vp[:, t, pc, 32] = valid
        kpt[:, t, 34, F + NPAST:] = NEG        # pad columns
    # bulk per-bh DMA layouts:
    #   kpt2[bh, r, t, c]        = kpt[bh, t, r, c]
    #   vp2[bh, p, pair, c, tb*VA+n] = vp[bh, 2*pair+tb, 128*c+p, n]
    kpt2 = np.ascontiguousarray(kpt.transpose(0, 2, 1, 3))
    vp2 = vp.reshape(nbh, T // 2, 2, 3, 128, VA).transpose(0, 4, 1, 3, 2, 5)
    vp2 = np.ascontiguousarray(vp2.reshape(nbh, 128, T // 2, 3, 2 * VA))
    return (qt.astype(NP_BF16), kpt2.astype(NP_BF16), vp2.astype(NP_BF16))


# ---------------------------------------------------------------- bass kernel

def build_nc(n_bh=BH_PER_CORE, n_pairs=NPAIR):
    nc = bacc.Bacc(None, target_bir_lowering=False, debug=False)
    qt_d = nc.declare_dram_parameter("qt", [BH_PER_CORE, KAUG, S], BF16, isOutput=False)
    kpt_d = nc.declare_dram_parameter("kpt", [BH_PER_CORE, KAUG, T, KV], BF16, isOutput=False)
    vp_d = nc.declare_dram_parameter("vp", [BH_PER_CORE, 128, T // 2, 3, 2 * VA], BF16, isOutput=False)
    out_d = nc.declare_dram_parameter("out", [BH_PER_CORE, S, D], F32, isOutput=True)

    def _strided2(ap, d1, d2):
        return bass.AP(tensor=ap.tensor, offset=ap.offset,
                       ap=[list(ap.ap[0]), list(d1), list(d2)])

    with tile.TileContext(nc) as tc:
        with (
            tc.tile_pool(name="singles", bufs=1) as singles,
            tc.tile_pool(name="qtp", bufs=3) as qtp,
            tc.tile_pool(name="kptp", bufs=3) as kptp,
            tc.tile_pool(name="vpp", bufs=3) as vpp,
            tc.tile_pool(name="probsp", bufs=3) as probsp,
            tc.tile_pool(name="pvsbp", bufs=3) as pvsbp,
            tc.tile_pool(name="recipsp", bufs=3) as recipsp,
            tc.tile_pool(name="outsbp", bufs=3) as outsbp,
            tc.tile_pool(name="scoresp", bufs=2, space="PSUM") as scoresp,
            tc.tile_pool(name="pvp", bufs=1, space="PSUM") as pvp,
        ):
            ident = singles.tile([128, 128], F32)
            make_identity(nc, ident[:])

            for i in range(n_bh):
                qt_sb = qtp.tile([KAUG, S], BF16)
                kpt_sb = kptp.tile([KAUG, T, KV], BF16)
                vp_sb = vpp.tile([128, T // 2, 3, 2 * VA], BF16)
                for hf in range(2):
                    hs, ts_, ps_ = S // 2 * hf, T // 2 * hf, NPAIR // 2 * hf
                    nc.sync.dma_start(out=qt_sb[:, hs:hs + S // 2],
                                      in_=qt_d[i, :, hs:hs + S // 2])
                    nc.sync.dma_start(out=kpt_sb[:, ts_:ts_ + T // 2, :],
                                      in_=kpt_d[i, :, ts_:ts_ + T // 2, :])
                    nc.sync.dma_start(
                        out=vp_sb[:, ps_:ps_ + NPAIR // 2, :, :],
                        in_=vp_d[i, :, ps_:ps_ + NPAIR // 2, :, :])
                outst = outsbp.tile([128, NPAIR * 128], F32)


                for p in range(n_pairs):
                    t0 = 2 * p

                    # ---- QK^T (transposed scores [kv, q]), mask via bias rows
                    scores = scoresp.tile([128, 1536], F32)   # 3 psum banks
                    for c in range(3):
                        first = None
                        for tb in range(2):
                            mm = nc.tensor.matmul(
                                scores[:, 512 * c + 216 * tb:512 * c + 216 * tb + 216],
                                lhsT=kpt_sb[:, t0 + tb, 128 * c:128 * c + 128],
                                rhs=qt_sb[:, 216 * (t0 + tb):216 * (t0 + tb) + 216],
                                start=(tb == 0), stop=(tb == 1))
                            if tb == 0:
                                first = mm
                            else:
                                add_dep_helper(mm.ins, first.ins, sync=False,
                                               reason="qk same-bank group order")

                    # ---- probs = exp(scale * scores), one ACT op
                    probs = probsp.tile([128, 1296], BF16)
                    sc_v = scores[:].rearrange("p (c x) -> p c x", c=3)[:, :, 0:432]
                    pr_v = probs[:].rearrange("p (c x) -> p c x", c=3)
                    nc.scalar.activation(pr_v, sc_v,
                                         mybir.ActivationFunctionType.Exp,
                                         scale=SCALE)

                    # ---- PV: out_T[va, q]; one psum bank per block
                    # (separate banks avoid interleaved-group pending-zero
                    # hazards and cross-engine bank overlap).
                    pv = pvp.tile([128, 1024], F32)
                    for tb in range(2):
                        for c in range(3):
                            nc.tensor.matmul(
                                pv[0:VA, 512 * tb:512 * tb + 216],
                                lhsT=vp_sb[:, p, c, VA * tb:VA * tb + VA],
                                rhs=probs[:, 432 * c + 216 * tb:432 * c + 216 * tb + 216],
                                start=(c == 0), stop=(c == 2))

                    # ---- evacuate both blocks to SBUF in one DVE op
                    pvsb = pvsbp.tile([VA, 2, 216], F32)
                    cp1 = nc.vector.tensor_copy(
                        pvsb[:],
                        _strided2(pv[0:VA, 0:1], (512, 2), (1, 216)))

                    # ---- PE-transpose each q-slice into the scores tile's
                    # spare columns (exp already consumed those banks; this
                    # frees the pv tile for the next pair right after cp1).
                    # qs=0 slots (128-wide) -> bank0 col 432+33*tb;
                    # qs=1 slots ( 88-wide) -> bank1 col 944+33*tb.
                    prevs = [None, None]
                    for tb in range(2):
                        for qs in range(2):
                            w = 128 if qs == 0 else 88
                            col = (432 if qs == 0 else 944) + 33 * tb
                            mm = nc.tensor.matmul(
                                scores[0:w, col:col + VA],
                                lhsT=pvsb[:, tb, 128 * qs:128 * qs + w],
                                rhs=ident[0:VA, 0:VA],
                                is_transpose=True,
                                start=(tb == 0), stop=(tb == 1))
                            if prevs[qs] is not None:
                                add_dep_helper(mm.ins, prevs[qs].ins,
                                               sync=False,
                                               reason="tr bank group order")
                            prevs[qs] = mm

                    # ---- normalize: out = num * (1/den), split by q-width
                    # (all reader->writer hazards here are RAW-tracked: each
                    # recip/mul reads both of its bank's transpose outputs)
                    recips = recipsp.tile([128, 4], F32)
                    nc.vector.reciprocal(recips[0:128, 0:3:2],
                                         scores[0:128, 464:498:33])
                    nc.vector.reciprocal(recips[0:88, 1:4:2],
                                         scores[0:88, 976:1010:33])
                    _strided = _strided2

                    for qs, w in ((0, 128), (1, 88)):
                        col = 432 if qs == 0 else 944
                        num_v = _strided(scores[0:w, col:col + 1],
                                         (33, 2), (1, 32))
                        rec_bcast = _strided(recips[0:w, qs:qs + 1],
                                             (2, 2), (0, 32))
                        out_v = _strided(
                            outst[0:w, 128 * p + 32 * qs:128 * p + 32 * qs + 1],
                            (64, 2), (1, 32))
                        nc.vector.tensor_mul(out_v, num_v, rec_bcast)

                    # ---- store half-(b,h) after pairs 0-3 / 4-7 complete
                    if p % (NPAIR // 2) == NPAIR // 2 - 1:
                        hf = p // (NPAIR // 2)
                        po = NPAIR // 2 * hf
                        for qs, w in ((0, 128), (1, 88)):
                            dst = bass.AP(
                                tensor=out_d,
                                offset=(i * S + 432 * po + 128 * qs) * D,
                                ap=[[D, w], [432 * D, NPAIR // 2],
                                    [216 * D, 2], [1, D]])
                            sap = outst[:]
                            sst = bass.AP(
                                tensor=sap.tensor,
                                offset=sap.offset + 128 * po + 32 * qs,
                                ap=[[sap.ap[0][0], w], [128, NPAIR // 2],
                                    [64, 2], [1, 32]])
                            nc.sync.dma_start(out=dst, in_=sst)
    nc.compile()
    return nc


_NC = None


def _get_nc():
    global _NC
    if _NC is None:
        _NC = build_nc()
    return _NC


# ---------------------------------------------------------------- entry point

def kernel(q, k, v, feats_per_t, window_len, act_size, img_feat_size):
    assert int(feats_per_t) == F and int(window_len) == W
    assert int(act_size) == 16 and int(img_feat_size) == 196
    q = np.asarray(q, np.float32)
    k = np.asarray(k, np.float32)
    v = np.asarray(v, np.float32)

    qt, kpt, vp = _pack_all(q, k, v)
    in_maps = []
    for core in range(N_CORES):
        s = slice(BH_PER_CORE * core, BH_PER_CORE * (core + 1))
        in_maps.append({"qt": np.ascontiguousarray(qt[s]),
                        "kpt": np.ascontiguousarray(kpt[s]),
                        "vp": np.ascontiguousarray(vp[s])})

    nc = _get_nc()
    res = run_bass_kernel_spmd(nc, in_maps, list(range(N_CORES)))
    out = np.empty((B * H, S, D), np.float32)
    for core in range(N_CORES):
        out[BH_PER_CORE * core:BH_PER_CORE * (core + 1)] = res.results[core]["out"]
    return out.reshape(B, H, S, D)



# revision 3
# speedup vs baseline: 1.5632x; 1.5632x over previous
"""Trainium2 Bass kernel for nn_EyeRobotAgent block-sparse ("eye") attention.

Shapes: q,k,v [2, 12, 3456, 32] fp32.  S = 16 time-blocks x 216 feats.
Mask structure (per query block t):
  - all 216 keys of block t are candidates (minus img->img),
  - of each past block t-7..t-1, only 19 keys (m in {0..3, 5..19}) are
    visible (proprio m==4 and img m>=20 keys are never visible in the past),
  - joint queries (m in [4,20)) cannot see past joint keys,
  - img queries (m >= 20) cannot see img keys at all.

Strategy (data-parallel: 24 (b,h) pairs over 8 cores, 3 each):
  Pack per block t a compact key set [216 same | 133 past | 35 pad] = 384.
  The 2-D mask folds into the QK matmul via 3 extra contraction rows
  (rank-1 decomposition of the mask predicates); invalid/pad columns get a
  large negative bias so exp() underflows to 0.  Scores are computed
  transposed [kv, q]; exp() runs as one ACT op per block-pair; the PV
  matmul consumes probs as the stationary operand producing out [q, 33]
  directly (numerators + denominator column), so no PE transposes or
  large PSUM->SBUF copies are needed.  Normalization (reciprocal + mul)
  reads PSUM directly on DVE.  Output DRAM tensor is fp16; the host
  casts to fp32 after gathering.
"""
import numpy as np

import concourse.bass as bass
import concourse.mybir as mybir
import concourse.tile as tile
from concourse import bacc
from concourse.bass_utils import run_bass_kernel_spmd

B, H, S, D = 2, 12, 3456, 32
F = 216            # feats_per_t
W = 8              # window_len
T = S // F         # 16 blocks
IMG_START = 20     # F - img_feat_size
JOINT_START = 4    # IMG_START - act_size
PAST_SEL = np.array([0, 1, 2, 3] + list(range(5, 20)))   # 19 per past block
NPAST = 19 * (W - 1)     # 133
KV = 384                 # packed kv per block (216 + 133 + pad)
KAUG = D + 3             # 35 contraction rows (32 d + 3 mask-bias rows)
VA = D + 1               # 33 = v columns + ones column
NEG = np.float32(-30000.0)
SCALE = float(1.0 / np.sqrt(np.float32(D)))
N_CORES = 8
BH_PER_CORE = (B * H) // N_CORES      # 3
NPAIR = T // 2                        # 8 block-pairs per (b,h)

F32 = mybir.dt.float32
BF16 = mybir.dt.float16
NP_BF16 = np.float16


# ---------------------------------------------------------------- host packing

def _pack_all(q, k, v):
    """q,k,v: [B,H,S,D] fp32 ->
       qt  [24, KAUG, S]       (augmented Q^T)
       kpt [24, KAUG, T, KV]   (augmented packed K^T per block)
       vp  [24, 128, T, 3, VA] (packed V + ones column per kv chunk)"""
    nbh = B * H
    qf = q.reshape(nbh, S, D)
    kf = k.reshape(nbh, S, D)
    vf = v.reshape(nbh, S, D)

    m = np.arange(F)
    is_img_m = (m >= IMG_START).astype(np.float32)
    is_joint_m = ((m >= JOINT_START) & (m < IMG_START)).astype(np.float32)
    qm = np.arange(S) % F

    qt = np.zeros((nbh, KAUG, S), np.float32)
    qt[:, :D] = qf.transpose(0, 2, 1)
    qt[:, 32] = (qm >= IMG_START)
    qt[:, 33] = (qm >= JOINT_START) & (qm < IMG_START)
    qt[:, 34] = 1.0

    kpt = np.zeros((nbh, T, KAUG, KV), np.float32)
    vp = np.zeros((nbh, T, KV, VA), np.float32)
    joint_past_bias = np.tile(NEG * is_joint_m[PAST_SEL], W - 1)  # [133]
    for t in range(T):
        blk = slice(F * t, F * (t + 1))
        kpt[:, t, :D, :F] = kf[:, blk].transpose(0, 2, 1)
        kpt[:, t, 32, :F] = NEG * is_img_m
        vp[:, t, :F, :D] = vf[:, blk]
        vp[:, t, :F, 32] = 1.0
        # past blocks t-7 .. t-1, 19 keys each
        taus = np.arange(t - 7, t)
        rows = (F * taus[:, None] + PAST_SEL[None, :]).reshape(-1)   # [133]
        valid = np.repeat(taus >= 0, 19)                             # [133]
        safe_rows = np.where(valid, rows, 0)
        pc = slice(F, F + NPAST)
        kpt[:, t, :D, pc] = np.where(
            valid[None, None, :], kf[:, safe_rows].transpose(0, 2, 1), 0.0)
        kpt[:, t, 33, pc] = joint_past_bias
        kpt[:, t, 34, pc] = np.where(valid, 0.0, NEG)
        vp[:, t, pc, :D] = np.where(
            valid[None, :, None], vf[:, safe_rows], 0.0)
        vp[:, t, pc, 32] = valid
        kpt[:, t, 34, F + NPAST:] = NEG        # pad columns
    # bulk per-bh DMA layouts:
    #   kpt2[bh, r, t, c]     = kpt[bh, t, r, c]
    #   vp2[bh, p, t, c, n]   = vp[bh, t, 128*c+p, n]
    kpt2 = np.ascontiguousarray(kpt.transpose(0, 2, 1, 3))
    vp2 = vp.reshape(nbh, T, 3, 128, VA).transpose(0, 3, 1, 2, 4)
    vp2 = np.ascontiguousarray(vp2)
    return (qt.astype(NP_BF16), kpt2.astype(NP_BF16), vp2.astype(NP_BF16))


# ---------------------------------------------------------------- bass kernel

def build_nc(n_bh=BH_PER_CORE, n_pairs=NPAIR):
    nc = bacc.Bacc(None, target_bir_lowering=False, debug=False)
    qt_d = nc.declare_dram_parameter("qt", [BH_PER_CORE, KAUG, S], BF16, isOutput=False)
    kpt_d = nc.declare_dram_parameter("kpt", [BH_PER_CORE, KAUG, T, KV], BF16, isOutput=False)
    vp_d = nc.declare_dram_parameter("vp", [BH_PER_CORE, 128, T, 3, VA], BF16, isOutput=False)
    out_d = nc.declare_dram_parameter("out", [BH_PER_CORE, S, D], BF16, isOutput=True)

    def _strided2(ap, d1, d2):
        return bass.AP(tensor=ap.tensor, offset=ap.offset,
                       ap=[list(ap.ap[0]), list(d1), list(d2)])

    with tile.TileContext(nc) as tc:
        with (
            tc.tile_pool(name="qtp", bufs=2) as qtp,
            tc.tile_pool(name="kptp", bufs=2) as kptp,
            tc.tile_pool(name="vpp", bufs=2) as vpp,
            tc.tile_pool(name="probsp", bufs=3) as probsp,
            tc.tile_pool(name="recipsp", bufs=3) as recipsp,
            tc.tile_pool(name="outsbp", bufs=2) as outsbp,
            tc.tile_pool(name="scoresp", bufs=2, space="PSUM") as scoresp,
            tc.tile_pool(name="pvp", bufs=2, space="PSUM") as pvp,
        ):
            for i in range(n_bh):
                qt_sb = qtp.tile([KAUG, S], BF16)
                kpt_sb = kptp.tile([KAUG, T, KV], BF16)
                vp_sb = vpp.tile([128, T, 3, VA], BF16)
                for hf in range(2):
                    hs, ts_ = S // 2 * hf, T // 2 * hf
                    # spread input loads over the SP and Pool queues
                    eng = nc.sync if hf == 0 else nc.gpsimd
                    nc.sync.dma_start(out=qt_sb[:, hs:hs + S // 2],
                                      in_=qt_d[i, :, hs:hs + S // 2])
                    eng.dma_start(out=kpt_sb[:, ts_:ts_ + T // 2, :],
                                  in_=kpt_d[i, :, ts_:ts_ + T // 2, :])
                    nc.gpsimd.dma_start(
                        out=vp_sb[:, ts_:ts_ + T // 2, :, :],
                        in_=vp_d[i, :, ts_:ts_ + T // 2, :, :])
                outst = outsbp.tile([128, NPAIR, 4, D], BF16)

                for p in range(n_pairs):
                    t0 = 2 * p

                    # ---- QK^T (transposed scores [kv, q]), mask via bias rows
                    scores = scoresp.tile([128, 1536], F32)   # 3 psum banks
                    for c in range(3):
                        for tb in range(2):
                            nc.tensor.matmul(
                                scores[:, 512 * c + 216 * tb:512 * c + 216 * tb + 216],
                                lhsT=kpt_sb[:, t0 + tb, 128 * c:128 * c + 128],
                                rhs=qt_sb[:, 216 * (t0 + tb):216 * (t0 + tb) + 216],
                                start=True, stop=True)

                    # ---- probs = exp(scale * scores), one ACT op
                    probs = probsp.tile([128, 1296], BF16)
                    sc_v = scores[:].rearrange("p (c x) -> p c x", c=3)[:, :, 0:432]
                    pr_v = probs[:].rearrange("p (c x) -> p c x", c=3)
                    nc.scalar.activation(pr_v, sc_v,
                                         mybir.ActivationFunctionType.Exp,
                                         scale=SCALE)

                    # ---- PV flipped: out[q, 33] = probs[kv, q]^T @ v[kv, 33]
                    # one psum bank, 4 col-groups g = 2*tb + qs
                    pv = pvp.tile([128, 4, VA], F32)
                    for tb in range(2):
                        for qs, w in ((0, 128), (1, 88)):
                            for c in range(3):
                                nc.tensor.matmul(
                                    pv[0:w, 2 * tb + qs, :],
                                    lhsT=probs[:, 432 * c + 216 * tb + 128 * qs:
                                               432 * c + 216 * tb + 128 * qs + w],
                                    rhs=vp_sb[:, t0 + tb, c, :],
                                    start=(c == 0), stop=(c == 2))

                    # ---- normalize on DVE straight from PSUM
                    # recips[:, g] = 1 / pv[:, g, 32]
                    recips = recipsp.tile([128, 4], F32)
                    pvf = pv[:].rearrange("p g x -> p (g x)")
                    nc.vector.reciprocal(recips[0:128, 0:3:2],
                                         _strided2(pvf[0:128, 32:33], (66, 2), (1, 1)))
                    nc.vector.reciprocal(recips[0:88, 1:4:2],
                                         _strided2(pvf[0:88, 65:66], (66, 2), (1, 1)))
                    for qs, w in ((0, 128), (1, 88)):
                        num_v = _strided2(pvf[0:w, 33 * qs:33 * qs + 1],
                                          (66, 2), (1, 32))
                        rec_b = _strided2(recips[0:w, qs:qs + 1], (2, 2), (0, 32))
                        out_v = _strided2(outst[0:w, p, qs, 0:1], (64, 2), (1, 32))
                        nc.vector.tensor_mul(out_v, num_v, rec_b)

                # ---- store per bh: 2 DMAs (qs=0 rows, qs=1 rows) on Pool
                for qs, w in ((0, 128), (1, 88)):
                    dst = bass.AP(
                        tensor=out_d,
                        offset=(i * S + 128 * qs) * D,
                        ap=[[D, w], [432 * D, NPAIR], [216 * D, 2], [1, D]])
                    sap = outst[:]
                    sst = bass.AP(
                        tensor=sap.tensor,
                        offset=sap.offset + qs * D,
                        ap=[[sap.ap[0][0], w], [4 * D, NPAIR], [2 * D, 2], [1, D]])
                    nc.gpsimd.dma_start(out=dst, in_=sst)
    nc.compile()
    return nc


_NC = None


def _get_nc():
    global _NC
    if _NC is None:
        _NC = build_nc()
    return _NC


# ---------------------------------------------------------------- entry point

def kernel(q, k, v, feats_per_t, window_len, act_size, img_feat_size):
    assert int(feats_per_t) == F and int(window_len) == W
    assert int(act_size) == 16 and int(img_feat_size) == 196
    q = np.asarray(q, np.float32)
    k = np.asarray(k, np.float32)
    v = np.asarray(v, np.float32)

    qt, kpt, vp = _pack_all(q, k, v)
    in_maps = []
    for core in range(N_CORES):
        s = slice(BH_PER_CORE * core, BH_PER_CORE * (core + 1))
        in_maps.append({"qt": np.ascontiguousarray(qt[s]),
                        "kpt": np.ascontiguousarray(kpt[s]),
                        "vp": np.ascontiguousarray(vp[s])})

    nc = _get_nc()
    res = run_bass_kernel_spmd(nc, in_maps, list(range(N_CORES)))
    out = np.empty((B * H, S, D), np.float32)
    for core in range(N_CORES):
        out[BH_PER_CORE * core:BH_PER_CORE * (core + 1)] = \
            res.results[core]["out"].astype(np.float32)
    return out.reshape(B, H, S, D)


# revision 8
# speedup vs baseline: 2.1153x; 1.3532x over previous
"""Trainium2 Bass kernel for nn_EyeRobotAgent block-sparse ("eye") attention.

Shapes: q,k,v [2, 12, 3456, 32] fp32.  S = 16 time-blocks x 216 feats.
Mask structure (per query block t):
  - all 216 keys of block t are candidates (minus img->img),
  - of each past block t-7..t-1, only 19 keys (m in {0..3, 5..19}) are
    visible (proprio m==4 and img m>=20 keys are never visible in the past),
  - joint queries (m in [4,20)) cannot see past joint keys,
  - img queries (m >= 20) cannot see img keys at all.

Strategy (data-parallel: 24 (b,h) pairs over 8 cores, 3 each).
Sparsity-aware score layout: img queries (196 of 216 per block) only see
153 kv (133 past + 20 same-block non-img), small queries (m 0..19) see
349.  Scores are computed transposed [kv, q] in 128-partition-exact
chunks, grouped 4 blocks (2 pairs) per PSUM tile so ONE exp() ACT op
covers 1240 columns.  Masking: joint-past via one augmented contraction
row (row32); invalid/pad kv need no mask at all because their V rows
and ones-column are zero (they contribute 0 to both numerator and
denominator).  32-row kv chunks stripe 4 blocks into one 128-partition
bank.  PV consumes probs as the stationary operand giving out [q, 33]
directly; normalization (reciprocal+mul) runs on DVE from PSUM.  The
DRAM output is a partition-major fp16 scratch layout; the host scatters
it back to [S, D] fp32 (free).
"""
import numpy as np

import concourse.bass as bass
import concourse.mybir as mybir
import concourse.tile as tile
from concourse import bacc
from concourse.bass_utils import run_bass_kernel_spmd

B, H, S, D = 2, 12, 3456, 32
F = 216            # feats_per_t
W = 8              # window_len
T = S // F         # 16 blocks
IMG_START = 20     # F - img_feat_size
NIMG = F - IMG_START   # 196 img queries per block
PAST_SEL = np.array([0, 1, 2, 3] + list(range(5, 20)))   # 19 per past block
NPAST = 19 * (W - 1)     # 133
KBLK = 388               # kall cols/block: 133 past |3 pad| 20 |36 pad| 196
VA = D + 1               # 33 = v columns + ones column
NEG = np.float32(-30000.0)
SCALE = float(1.0 / np.sqrt(np.float32(D)))
N_CORES = 8
BH_PER_CORE = (B * H) // N_CORES      # 3
NGRP = T // 4                         # 4 groups of 4 blocks per (b,h)

F32 = mybir.dt.float32
FP16 = mybir.dt.float16
NP_FP16 = np.float16

# scores col layout per 4-block group: 3 PSUM banks (512 fp32 cols each),
# every matmul output region within one bank (2x196 + 6x20 = 512 exact).
COL_IMG0 = (0, 196, 512, 708)     # [j]  img q x past[0:128]
COL_IMG1 = (1024, 1220)           # [jc] img q x c1-64 chunk (64-row stripes)
COL_SM0 = (392, 412, 432, 452)    # [j]  small q x past[0:128]
COL_SM1 = (472, 492)              # [jc] small q x c1-64 chunk
COL_SM2 = (904, 924, 944, 964)    # [j]  small q x same m20..147
COL_SM3 = (984, 1004, 1416, 1436)  # [j] small q x same m148..215 (+overflow)
NCOL = 1456


# ---------------------------------------------------------------- host packing

def _pack_all(q, k, v):
    nbh = B * H
    qf = q.reshape(nbh, S, D).astype(np.float32)
    kf = k.reshape(nbh, S, D).astype(np.float32)
    vf = v.reshape(nbh, S, D).astype(np.float32)

    is_joint = lambda m: (m >= 4) & (m < IMG_START)

    # qaug [nbh, 33, S]: rows 0..31 q^T, row32 = is_joint(s % F)
    qaug = np.empty((nbh, 33, S), np.float32)
    qaug[:, :D] = qf.transpose(0, 2, 1)
    qaug[:, 32] = is_joint(np.arange(S) % F).astype(np.float32)

    # kall [nbh, 33, 16*356 + 60]
    kall = np.zeros((nbh, 33, T * KBLK + 60), np.float32)
    # vall [nbh, 128, T, 4, 33]
    vall = np.zeros((nbh, 128, T, 4, VA), np.float32)

    sel_m = np.tile(PAST_SEL, W - 1)                      # [133] m of past idx
    sel_tau_off = np.repeat(np.arange(-7, 0), 19)         # [133] tau - t
    joint_bias = NEG * is_joint(sel_m).astype(np.float32)  # [133]

    for t in range(T):
        base = KBLK * t
        taus = t + sel_tau_off
        valid = taus >= 0
        rows = np.where(valid, F * taus + sel_m, 0)
        kpast = np.where(valid[None, None, :], kf[:, rows].transpose(0, 2, 1), 0.0)
        # past cols
        kall[:, :D, base:base + NPAST] = kpast
        kall[:, 32, base:base + NPAST] = joint_bias
        # nonimg 20 (m 0..19 of block t) at cols base+136..155
        kall[:, :D, base + 136:base + 156] = \
            kf[:, F * t:F * t + IMG_START].transpose(0, 2, 1)
        # same-img 196 at cols base+192..387
        kall[:, :D, base + 192:base + KBLK] = \
            kf[:, F * t + IMG_START:F * (t + 1)].transpose(0, 2, 1)

        vpast = np.where(valid[None, :, None], vf[:, rows], 0.0)  # [nbh,133,32]
        ones_v = valid.astype(np.float32)
        # c0: past idx 0..127
        vall[:, :, t, 0, :D] = vpast[:, :128]
        vall[:, :, t, 0, 32] = ones_v[:128]
        # c1: stripe at partitions 64*(j%2): [past 128:133 |0x3| m0..19 |0x4]
        j = t % 4
        sb = 64 * (j % 2)
        sl = slice(sb, sb + 5)
        vall[:, sl, t, 1, :D] = vpast[:, 128:133]
        vall[:, sl, t, 1, 32] = ones_v[128:133]
        sl2 = slice(sb + 8, sb + 28)
        vall[:, sl2, t, 1, :D] = vf[:, F * t:F * t + IMG_START]
        vall[:, sl2, t, 1, 32] = 1.0
        # c2: same m20..147
        vall[:, :, t, 2, :D] = vf[:, F * t + 20:F * t + 148]
        vall[:, :, t, 2, 32] = 1.0
        # c3: same m148..215 at partitions 0..67
        vall[:, :68, t, 3, :D] = vf[:, F * t + 148:F * (t + 1)]
        vall[:, :68, t, 3, 32] = 1.0

    return {"qaug": qaug.astype(NP_FP16),
            "kall": kall.astype(NP_FP16),
            "vall": np.ascontiguousarray(vall).astype(NP_FP16)}


def _unpack_out(r0, r1, r2):
    """r0 [nbh,128,T,32], r1 [nbh,68,T,32], r2 [nbh,20,T,32] fp16 -> fp32"""
    nbh = r0.shape[0]
    out = np.empty((nbh, S, D), np.float32)
    for t in range(T):
        out[:, F * t + 20:F * t + 148] = r0[:, :, t]
        out[:, F * t + 148:F * (t + 1)] = r1[:, :, t]
        out[:, F * t:F * t + 20] = r2[:, :, t]
    return out


# ---------------------------------------------------------------- bass kernel

def build_nc(n_bh=BH_PER_CORE):
    nc = bacc.Bacc(None, target_bir_lowering=False, debug=False)
    qaug_d = nc.declare_dram_parameter("qaug", [BH_PER_CORE, 33, S], FP16, isOutput=False)
    kall_d = nc.declare_dram_parameter("kall", [BH_PER_CORE, 33, T * KBLK + 60], FP16, isOutput=False)
    vall_d = nc.declare_dram_parameter("vall", [BH_PER_CORE, 128, T, 4, VA], FP16, isOutput=False)
    out_d0 = nc.declare_dram_parameter("out0", [BH_PER_CORE, 128, T, D], FP16, isOutput=True)
    out_d1 = nc.declare_dram_parameter("out1", [BH_PER_CORE, 68, T, D], FP16, isOutput=True)
    out_d2 = nc.declare_dram_parameter("out2", [BH_PER_CORE, 20, T, D], FP16, isOutput=True)

    def _str2(ap, d1, d2):
        return bass.AP(tensor=ap.tensor, offset=ap.offset,
                       ap=[list(ap.ap[0]), list(d1), list(d2)])

    with tile.TileContext(nc) as tc:
        with (
            tc.tile_pool(name="qp", bufs=2) as qp,
            tc.tile_pool(name="kp", bufs=2) as kp,
            tc.tile_pool(name="vp", bufs=2) as vp,
            tc.tile_pool(name="probsp", bufs=3) as probsp,
            tc.tile_pool(name="recipsp", bufs=3) as recipsp,
            tc.tile_pool(name="outsbp", bufs=2) as outsbp,
            tc.tile_pool(name="scoresp", bufs=2, space="PSUM") as scoresp,
            tc.tile_pool(name="pvp", bufs=2, space="PSUM") as pvp,
        ):
            for i in range(n_bh):
                q_sb = qp.tile([33, S], FP16)
                k_sb = kp.tile([33, T * KBLK + 60], FP16)
                v_sb = vp.tile([128, T, 4, VA], FP16)
                nc.gpsimd.dma_start(out=q_sb[:], in_=qaug_d[i])
                nc.sync.dma_start(out=k_sb[:], in_=kall_d[i])
                nc.gpsimd.dma_start(out=v_sb[:], in_=vall_d[i])
                o0 = outsbp.tile([128, T, D], FP16)
                o1 = outsbp.tile([68, T, D], FP16)
                o2 = outsbp.tile([20, T, D], FP16)

                for g in range(NGRP):
                    # ---------------- QK^T: scores [kv, q], 24 matmuls
                    sc = scoresp.tile([128, NCOL], F32)
                    for j in range(4):
                        t = 4 * g + j
                        kb = KBLK * t
                        sb = 64 * (j % 2)
                        jc = j // 2
                        qi = q_sb[0:32, F * t + 20:F * (t + 1)]
                        qs = q_sb[0:33, F * t:F * t + 20]
                        # img q x past[0:128]
                        nc.tensor.matmul(
                            sc[0:128, COL_IMG0[j]:COL_IMG0[j] + NIMG],
                            lhsT=k_sb[0:32, kb:kb + 128], rhs=qi,
                            start=True, stop=True)
                        # img q x c1-64 [past 5|pad|nonimg 20|pad36] (64-row stripe)
                        nc.tensor.matmul(
                            sc[sb:sb + 64, COL_IMG1[jc]:COL_IMG1[jc] + NIMG],
                            lhsT=k_sb[0:32, kb + 128:kb + 192], rhs=qi,
                            start=True, stop=True)
                        # small q x past[0:128]
                        nc.tensor.matmul(
                            sc[0:128, COL_SM0[j]:COL_SM0[j] + 20],
                            lhsT=k_sb[0:33, kb:kb + 128], rhs=qs,
                            start=True, stop=True)
                        # small q x c1-64 (64-row stripe)
                        nc.tensor.matmul(
                            sc[sb:sb + 64, COL_SM1[jc]:COL_SM1[jc] + 20],
                            lhsT=k_sb[0:33, kb + 128:kb + 192], rhs=qs,
                            start=True, stop=True)
                        # small q x same m20..147
                        nc.tensor.matmul(
                            sc[0:128, COL_SM2[j]:COL_SM2[j] + 20],
                            lhsT=k_sb[0:33, kb + 192:kb + 320], rhs=qs,
                            start=True, stop=True)
                        # small q x same m148..215 (+60 overflow cols, v-zeroed)
                        nc.tensor.matmul(
                            sc[0:128, COL_SM3[j]:COL_SM3[j] + 20],
                            lhsT=k_sb[0:33, kb + 320:kb + 448], rhs=qs,
                            start=True, stop=True)

                    # ---------------- probs = exp(scale * scores): ONE ACT op
                    probs = probsp.tile([128, NCOL], FP16)
                    nc.scalar.activation(probs[:], sc[0:128, 0:NCOL],
                                         mybir.ActivationFunctionType.Exp,
                                         scale=SCALE)

                    # ---------------- PV: out[q, 33] per block, 32 matmuls
                    pv = pvp.tile([128, 12, VA], F32)
                    for j in range(4):
                        t = 4 * g + j
                        sb = 64 * (j % 2)
                        jc = j // 2
                        st = slice(sb, sb + 32)
                        # img q m20..147 -> group 3j; m148..215 -> group 3j+1
                        for sub, off, w in ((0, 0, 128), (1, 128, 68)):
                            nc.tensor.matmul(
                                pv[0:w, 3 * j + sub, :],
                                lhsT=probs[0:128, COL_IMG0[j] + off:
                                           COL_IMG0[j] + off + w],
                                rhs=v_sb[0:128, t, 0, :],
                                start=True, stop=False)
                            nc.tensor.matmul(
                                pv[0:w, 3 * j + sub, :],
                                lhsT=probs[st, COL_IMG1[jc] + off:
                                           COL_IMG1[jc] + off + w],
                                rhs=v_sb[st, t, 1, :],
                                start=False, stop=True)
                        # small q -> group 3j+2 at partitions 0..19
                        for ci, (col, vc, kpart) in enumerate((
                                (COL_SM0[j], 0, slice(0, 128)),
                                (COL_SM1[jc], 1, st),
                                (COL_SM2[j], 2, slice(0, 128)),
                                (COL_SM3[j], 3, slice(0, 128)))):
                            nc.tensor.matmul(
                                pv[0:20, 3 * j + 2, :],
                                lhsT=probs[kpart, col:col + 20],
                                rhs=v_sb[kpart, t, vc, :],
                                start=(ci == 0), stop=(ci == 3))

                    # ---------------- normalize (DVE, straight from PSUM)
                    recips = recipsp.tile([128, 12], F32)
                    pvf = pv[:].rearrange("p g x -> p (g x)")
                    nc.vector.reciprocal(recips[0:128, 0:4],
                                         _str2(pvf[0:128, 32:33], (99, 4), (1, 1)))
                    nc.vector.reciprocal(recips[0:68, 4:8],
                                         _str2(pvf[0:68, 65:66], (99, 4), (1, 1)))
                    nc.vector.reciprocal(recips[0:20, 8:12],
                                         _str2(pvf[0:20, 98:99], (99, 4), (1, 1)))
                    for (ot, phi, sub) in ((o0, 128, 0), (o1, 68, 1), (o2, 20, 2)):
                        num_v = _str2(pvf[0:phi, 33 * sub:33 * sub + 1],
                                      (99, 4), (1, 32))
                        rec_b = _str2(recips[0:phi, 4 * sub:4 * sub + 1],
                                      (1, 4), (0, 32))
                        out_v = _str2(ot[0:phi, 4 * g, 0:1], (D, 4), (1, 32))
                        nc.vector.tensor_mul(out_v, num_v, rec_b)

                # ---- contiguous partition-major stores per bh
                nc.sync.dma_start(out=out_d0[i], in_=o0[:])
                nc.sync.dma_start(out=out_d1[i], in_=o1[:])
                nc.gpsimd.dma_start(out=out_d2[i], in_=o2[:])
    nc.compile()
    return nc


_NC = None


def _get_nc():
    global _NC
    if _NC is None:
        _NC = build_nc()
    return _NC


# ---------------------------------------------------------------- entry point

def kernel(q, k, v, feats_per_t, window_len, act_size, img_feat_size):
    assert int(feats_per_t) == F and int(window_len) == W
    assert int(act_size) == 16 and int(img_feat_size) == 196

    packed = _pack_all(np.asarray(q, np.float32), np.asarray(k, np.float32),
                       np.asarray(v, np.float32))
    in_maps = []
    for core in range(N_CORES):
        s = slice(BH_PER_CORE * core, BH_PER_CORE * (core + 1))
        in_maps.append({n: np.ascontiguousarray(a[s])
                        for n, a in packed.items()})

    nc = _get_nc()
    res = run_bass_kernel_spmd(nc, in_maps, list(range(N_CORES)))
    out = np.empty((B * H, S, D), np.float32)
    for core in range(N_CORES):
        r = res.results[core]
        out[BH_PER_CORE * core:BH_PER_CORE * (core + 1)] = \
            _unpack_out(r["out0"].astype(np.float32),
                        r["out1"].astype(np.float32),
                        r["out2"].astype(np.float32))
    return out.reshape(B, H, S, D)


# revision 9
# speedup vs baseline: 2.5423x; 1.2019x over previous
"""Trainium2 Bass kernel for nn_EyeRobotAgent block-sparse ("eye") attention.

Shapes: q,k,v [2, 12, 3456, 32] fp32.  S = 16 time-blocks x 216 feats.
Mask structure (per query block t):
  - all 216 keys of block t are candidates (minus img->img),
  - of each past block t-7..t-1, only 19 keys (m in {0..3, 5..19}) are
    visible (proprio m==4 and img m>=20 keys are never visible in the past),
  - joint queries (m in [4,20)) cannot see past joint keys,
  - img queries (m >= 20) cannot see img keys at all.

Strategy (data-parallel: 24 (b,h) pairs over 8 cores, 3 each).
Sparsity-aware score layout: img queries (196 of 216 per block) only see
153 kv (133 past + 20 same-block non-img), small queries (m 0..19) see
349.  Scores are computed transposed [kv, q] in 128-partition-exact
chunks, grouped 4 blocks (2 pairs) per PSUM tile so ONE exp() ACT op
covers 1240 columns.  Masking: joint-past via one augmented contraction
row (row32); invalid/pad kv need no mask at all because their V rows
and ones-column are zero (they contribute 0 to both numerator and
denominator).  32-row kv chunks stripe 4 blocks into one 128-partition
bank.  PV consumes probs as the stationary operand giving out [q, 33]
directly; normalization (reciprocal+mul) runs on DVE from PSUM.  The
DRAM output is a partition-major fp16 scratch layout; the host scatters
it back to [S, D] fp32 (free).
"""
import numpy as np

import concourse.bass as bass
import concourse.mybir as mybir
import concourse.tile as tile
from concourse import bacc
from concourse.bass_utils import run_bass_kernel_spmd

B, H, S, D = 2, 12, 3456, 32
F = 216            # feats_per_t
W = 8              # window_len
T = S // F         # 16 blocks
IMG_START = 20     # F - img_feat_size
NIMG = F - IMG_START   # 196 img queries per block
PAST_SEL = np.array([0, 1, 2, 3] + list(range(5, 20)))   # 19 per past block
NPAST = 19 * (W - 1)     # 133
KBLK = 388               # kall cols/block: 133 past |3 pad| 20 |36 pad| 196
VA = D + 1               # 33 = v columns + ones column
NEG = np.float32(-30000.0)
SCALE = float(1.0 / np.sqrt(np.float32(D)))
N_CORES = 8
BH_PER_CORE = (B * H) // N_CORES      # 3
NGRP = T // 4                         # 4 groups of 4 blocks per (b,h)

F32 = mybir.dt.float32
FP16 = mybir.dt.float16
NP_FP16 = np.float16

# scores col layout per 4-block group: 3 PSUM banks (512 fp32 cols each),
# every matmul output region within one bank (2x196 + 6x20 = 512 exact).
COL_IMG0 = (0, 196, 512, 708)     # [j]  img q x past[0:128]
COL_IMG1 = (1024, 1220)           # [jc] img q x c1-64 chunk (64-row stripes)
COL_SM0 = (392, 412, 432, 452)    # [j]  small q x past[0:128]
COL_SM1 = (472, 492)              # [jc] small q x c1-64 chunk
COL_SM2 = (904, 924, 944, 964)    # [j]  small q x same m20..147
COL_SM3 = (984, 1004, 1416, 1436)  # [j] small q x same m148..215 (+overflow)
NCOL = 1456


# ---------------------------------------------------------------- host packing

def _pack_all(q, k, v):
    nbh = B * H
    qf = q.reshape(nbh, S, D).astype(np.float32)
    kf = k.reshape(nbh, S, D).astype(np.float32)
    vf = v.reshape(nbh, S, D).astype(np.float32)

    is_joint = lambda m: (m >= 4) & (m < IMG_START)

    # qaug [nbh, 33, S]: rows 0..31 q^T, row32 = is_joint(s % F)
    qaug = np.empty((nbh, 33, S), np.float32)
    qaug[:, :D] = qf.transpose(0, 2, 1)
    qaug[:, 32] = is_joint(np.arange(S) % F).astype(np.float32)

    # kall [nbh, 33, 16*356 + 60]
    kall = np.zeros((nbh, 33, T * KBLK + 60), np.float32)
    # vall [nbh, 128, T, 4, 33]
    vall = np.zeros((nbh, 128, T, 4, VA), np.float32)

    sel_m = np.tile(PAST_SEL, W - 1)                      # [133] m of past idx
    sel_tau_off = np.repeat(np.arange(-7, 0), 19)         # [133] tau - t
    joint_bias = NEG * is_joint(sel_m).astype(np.float32)  # [133]

    for t in range(T):
        base = KBLK * t
        taus = t + sel_tau_off
        valid = taus >= 0
        rows = np.where(valid, F * taus + sel_m, 0)
        kpast = np.where(valid[None, None, :], kf[:, rows].transpose(0, 2, 1), 0.0)
        # past cols
        kall[:, :D, base:base + NPAST] = kpast
        kall[:, 32, base:base + NPAST] = joint_bias
        # nonimg 20 (m 0..19 of block t) at cols base+136..155
        kall[:, :D, base + 136:base + 156] = \
            kf[:, F * t:F * t + IMG_START].transpose(0, 2, 1)
        # same-img 196 at cols base+192..387
        kall[:, :D, base + 192:base + KBLK] = \
            kf[:, F * t + IMG_START:F * (t + 1)].transpose(0, 2, 1)

        vpast = np.where(valid[None, :, None], vf[:, rows], 0.0)  # [nbh,133,32]
        ones_v = valid.astype(np.float32)
        # c0: past idx 0..127
        vall[:, :, t, 0, :D] = vpast[:, :128]
        vall[:, :, t, 0, 32] = ones_v[:128]
        # c1: stripe at partitions 64*(j%2): [past 128:133 |0x3| m0..19 |0x4]
        j = t % 4
        sb = 64 * (j % 2)
        sl = slice(sb, sb + 5)
        vall[:, sl, t, 1, :D] = vpast[:, 128:133]
        vall[:, sl, t, 1, 32] = ones_v[128:133]
        sl2 = slice(sb + 8, sb + 28)
        vall[:, sl2, t, 1, :D] = vf[:, F * t:F * t + IMG_START]
        vall[:, sl2, t, 1, 32] = 1.0
        # c2: same m20..147
        vall[:, :, t, 2, :D] = vf[:, F * t + 20:F * t + 148]
        vall[:, :, t, 2, 32] = 1.0
        # c3: same m148..215 at partitions 0..67
        vall[:, :68, t, 3, :D] = vf[:, F * t + 148:F * (t + 1)]
        vall[:, :68, t, 3, 32] = 1.0

    return {"qaug": qaug.astype(NP_FP16),
            "kall": kall.astype(NP_FP16),
            "vall": np.ascontiguousarray(vall).astype(NP_FP16)}


def _unpack_out(r0, r1, r2):
    """r0 [nbh,128,T,32], r1 [nbh,68,T,32], r2 [nbh,20,T,32] fp16 -> fp32"""
    nbh = r0.shape[0]
    out = np.empty((nbh, S, D), np.float32)
    for t in range(T):
        out[:, F * t + 20:F * t + 148] = r0[:, :, t]
        out[:, F * t + 148:F * (t + 1)] = r1[:, :, t]
        out[:, F * t:F * t + 20] = r2[:, :, t]
    return out


# ---------------------------------------------------------------- bass kernel

def build_nc(n_bh=BH_PER_CORE):
    nc = bacc.Bacc(None, target_bir_lowering=False, debug=False)
    qaug_d = nc.declare_dram_parameter("qaug", [BH_PER_CORE, 33, S], FP16, isOutput=False)
    kall_d = nc.declare_dram_parameter("kall", [BH_PER_CORE, 33, T * KBLK + 60], FP16, isOutput=False)
    vall_d = nc.declare_dram_parameter("vall", [BH_PER_CORE, 128, T, 4, VA], FP16, isOutput=False)
    out_d0 = nc.declare_dram_parameter("out0", [BH_PER_CORE, 128, T, D], FP16, isOutput=True)
    out_d1 = nc.declare_dram_parameter("out1", [BH_PER_CORE, 68, T, D], FP16, isOutput=True)
    out_d2 = nc.declare_dram_parameter("out2", [BH_PER_CORE, 20, T, D], FP16, isOutput=True)

    def _str2(ap, d1, d2):
        return bass.AP(tensor=ap.tensor, offset=ap.offset,
                       ap=[list(ap.ap[0]), list(d1), list(d2)])

    with tile.TileContext(nc) as tc:
        with (
            tc.tile_pool(name="qp", bufs=2) as qp,
            tc.tile_pool(name="kp", bufs=2) as kp,
            tc.tile_pool(name="vp", bufs=2) as vp,
            tc.tile_pool(name="probsp", bufs=3) as probsp,
            tc.tile_pool(name="recipsp", bufs=3) as recipsp,
            tc.tile_pool(name="outsbp", bufs=2) as outsbp,
            tc.tile_pool(name="scoresp", bufs=2, space="PSUM") as scoresp,
            tc.tile_pool(name="pvp", bufs=2, space="PSUM") as pvp,
        ):
            # warm the Exp activation table while the first loads run
            scratch = qp.tile([1, 4], F32)
            nc.gpsimd.memset(scratch[:], 0.0)
            nc.scalar.activation(scratch[:], scratch[:],
                                 mybir.ActivationFunctionType.Exp, scale=1.0)

            K4 = 4 * KBLK
            for i in range(n_bh):
                q_sb = qp.tile([33, S], FP16)
                k_sb = kp.tile([33, T * KBLK + 60], FP16)
                v_sb = vp.tile([128, T, 4, VA], FP16)
                # group-granular load chunks so group 0 can start early
                kc = [0, K4 + 60, 2 * K4 + 60, 3 * K4 + 60, 4 * K4 + 60]
                for g in range(4):
                    nc.sync.dma_start(out=k_sb[:, kc[g]:kc[g + 1]],
                                      in_=kall_d[i, :, kc[g]:kc[g + 1]])
                    nc.gpsimd.dma_start(
                        out=q_sb[:, 864 * g:864 * (g + 1)],
                        in_=qaug_d[i, :, 864 * g:864 * (g + 1)])
                    if g % 2 == 0:
                        nc.gpsimd.dma_start(
                            out=v_sb[:, 4 * g:4 * g + 8, :, :],
                            in_=vall_d[i, :, 4 * g:4 * g + 8, :, :])
                o0 = outsbp.tile([128, T, D], FP16)
                o1 = outsbp.tile([68, T, D], FP16)
                o2 = outsbp.tile([20, T, D], FP16)

                for g in range(NGRP):
                    # ---------------- QK^T: scores [kv, q], 24 matmuls
                    sc = scoresp.tile([128, NCOL], F32)
                    for j in range(4):
                        t = 4 * g + j
                        kb = KBLK * t
                        sb = 64 * (j % 2)
                        jc = j // 2
                        qi = q_sb[0:32, F * t + 20:F * (t + 1)]
                        qs = q_sb[0:33, F * t:F * t + 20]
                        # img q x past[0:128]
                        nc.tensor.matmul(
                            sc[0:128, COL_IMG0[j]:COL_IMG0[j] + NIMG],
                            lhsT=k_sb[0:32, kb:kb + 128], rhs=qi,
                            start=True, stop=True)
                        # img q x c1-64 [past 5|pad|nonimg 20|pad36] (64-row stripe)
                        nc.tensor.matmul(
                            sc[sb:sb + 64, COL_IMG1[jc]:COL_IMG1[jc] + NIMG],
                            lhsT=k_sb[0:32, kb + 128:kb + 192], rhs=qi,
                            start=True, stop=True)
                        # small q x past[0:128]
                        nc.tensor.matmul(
                            sc[0:128, COL_SM0[j]:COL_SM0[j] + 20],
                            lhsT=k_sb[0:33, kb:kb + 128], rhs=qs,
                            start=True, stop=True)
                        # small q x c1-64 (64-row stripe)
                        nc.tensor.matmul(
                            sc[sb:sb + 64, COL_SM1[jc]:COL_SM1[jc] + 20],
                            lhsT=k_sb[0:33, kb + 128:kb + 192], rhs=qs,
                            start=True, stop=True)
                        # small q x same m20..147
                        nc.tensor.matmul(
                            sc[0:128, COL_SM2[j]:COL_SM2[j] + 20],
                            lhsT=k_sb[0:33, kb + 192:kb + 320], rhs=qs,
                            start=True, stop=True)
                        # small q x same m148..215 (+60 overflow cols, v-zeroed)
                        nc.tensor.matmul(
                            sc[0:128, COL_SM3[j]:COL_SM3[j] + 20],
                            lhsT=k_sb[0:33, kb + 320:kb + 448], rhs=qs,
                            start=True, stop=True)

                    # ---------------- probs = exp(scale * scores): ONE ACT op
                    probs = probsp.tile([128, NCOL], FP16)
                    nc.scalar.activation(probs[:], sc[0:128, 0:NCOL],
                                         mybir.ActivationFunctionType.Exp,
                                         scale=SCALE)

                    # ---------------- PV: out[q, 33] per block, 32 matmuls
                    pv = pvp.tile([128, 12, VA], F32)
                    for j in range(4):
                        t = 4 * g + j
                        sb = 64 * (j % 2)
                        jc = j // 2
                        st = slice(sb, sb + 32)
                        # img q m20..147 -> group 3j; m148..215 -> group 3j+1
                        for sub, off, w in ((0, 0, 128), (1, 128, 68)):
                            nc.tensor.matmul(
                                pv[0:w, 3 * j + sub, :],
                                lhsT=probs[0:128, COL_IMG0[j] + off:
                                           COL_IMG0[j] + off + w],
                                rhs=v_sb[0:128, t, 0, :],
                                start=True, stop=False)
                            nc.tensor.matmul(
                                pv[0:w, 3 * j + sub, :],
                                lhsT=probs[st, COL_IMG1[jc] + off:
                                           COL_IMG1[jc] + off + w],
                                rhs=v_sb[st, t, 1, :],
                                start=False, stop=True)
                        # small q -> group 3j+2 at partitions 0..19
                        for ci, (col, vc, kpart) in enumerate((
                                (COL_SM0[j], 0, slice(0, 128)),
                                (COL_SM1[jc], 1, st),
                                (COL_SM2[j], 2, slice(0, 128)),
                                (COL_SM3[j], 3, slice(0, 128)))):
                            nc.tensor.matmul(
                                pv[0:20, 3 * j + 2, :],
                                lhsT=probs[kpart, col:col + 20],
                                rhs=v_sb[kpart, t, vc, :],
                                start=(ci == 0), stop=(ci == 3))

                    # ---------------- normalize (DVE, straight from PSUM)
                    recips = recipsp.tile([128, 12], F32)
                    pvf = pv[:].rearrange("p g x -> p (g x)")
                    nc.vector.reciprocal(recips[0:128, 0:4],
                                         _str2(pvf[0:128, 32:33], (99, 4), (1, 1)))
                    nc.vector.reciprocal(recips[0:68, 4:8],
                                         _str2(pvf[0:68, 65:66], (99, 4), (1, 1)))
                    nc.vector.reciprocal(recips[0:20, 8:12],
                                         _str2(pvf[0:20, 98:99], (99, 4), (1, 1)))
                    for (ot, phi, sub) in ((o0, 128, 0), (o1, 68, 1), (o2, 20, 2)):
                        num_v = _str2(pvf[0:phi, 33 * sub:33 * sub + 1],
                                      (99, 4), (1, 32))
                        rec_b = _str2(recips[0:phi, 4 * sub:4 * sub + 1],
                                      (1, 4), (0, 32))
                        out_v = _str2(ot[0:phi, 4 * g, 0:1], (D, 4), (1, 32))
                        nc.vector.tensor_mul(out_v, num_v, rec_b)

                # ---- contiguous partition-major stores per bh
                nc.sync.dma_start(out=out_d0[i], in_=o0[:])
                nc.gpsimd.dma_start(out=out_d1[i], in_=o1[:])
                nc.gpsimd.dma_start(out=out_d2[i], in_=o2[:])
    nc.compile()
    return nc


_NC = None


def _get_nc():
    global _NC
    if _NC is None:
        _NC = build_nc()
    return _NC


# ---------------------------------------------------------------- entry point

def kernel(q, k, v, feats_per_t, window_len, act_size, img_feat_size):
    assert int(feats_per_t) == F and int(window_len) == W
    assert int(act_size) == 16 and int(img_feat_size) == 196

    packed = _pack_all(np.asarray(q, np.float32), np.asarray(k, np.float32),
                       np.asarray(v, np.float32))
    in_maps = []
    for core in range(N_CORES):
        s = slice(BH_PER_CORE * core, BH_PER_CORE * (core + 1))
        in_maps.append({n: np.ascontiguousarray(a[s])
                        for n, a in packed.items()})

    nc = _get_nc()
    res = run_bass_kernel_spmd(nc, in_maps, list(range(N_CORES)))
    out = np.empty((B * H, S, D), np.float32)
    for core in range(N_CORES):
        r = res.results[core]
        out[BH_PER_CORE * core:BH_PER_CORE * (core + 1)] = \
            _unpack_out(r["out0"].astype(np.float32),
                        r["out1"].astype(np.float32),
                        r["out2"].astype(np.float32))
    return out.reshape(B, H, S, D)
